# revision 1
# baseline (speedup 1.0000x reference)
"""MAPE loss on 8 Trainium2 NeuronCores (raw Bass, software-pipelined).

MAPE = mean(|pred - label| / label) * 100 over 2**25 f32 elements.

Sharding: pure data parallel. Each of the 8 cores gets a contiguous 1/8
slice of both tensors (4,194,304 elements = 16 MiB per tensor per core,
32 MiB of HBM reads per core -> memory-bound, roofline ~94 us at
~358 GB/s per-NC HBM bandwidth).

Per core, per [128, F] f32 tile (pipelined, BUFS slots; default builder
is build_nc_dual, which splits the two input streams across both HWDGE
rings -- x via the SP sequencer, y issued from the ACT stream):
  SP ring:  DMA x tile into SBUF slot s
  ACT ring: DMA y tile into SBUF slot s
  ACT:      y <- Reciprocal(y)        (table act, ~1e-6 mean rel err)
  DVE:      x <- x * y                (tensor_tensor mult)
  ACT:      acc[:, i] = sum_f |x - 1| (Abs activation with accum_out)
Per-partition partial sums [128, NT] are DMA'd out per core; the final
mean is reduced on the host in float64.

Measured (marginal-R wall-clock, see bench.py): ~95-115 us/core per full
pass depending on system load, equal within noise to a DMA-only probe of
the same traffic -- i.e. at the HBM streaming floor (~94 us theoretical
at 358 GB/s per NC). Compute is fully hidden behind the DMA stream.

|x/y - 1| == |x - y| / y exactly, since y > 0 (labels in (1e-3, 1)).

Raw Bass (not Tile): the Tile kernel-tail drain emits multi-wait CTRL
instructions this walrus build rejects ("Too many sync wait commands"),
and custom-DVE / ISA-class ops ("ISA wrong length") don't compile either.
Semaphore discipline:
  - sem_load[s] (one per buffer slot): +16 per DMA; tile k's loads are
    complete iff sem_load[k%B] >= 32*(k//B+1). Per-slot sems are needed
    because HWDGE completions across different tiles are not ordered.
  - rsem/msem/asem: recip/mult/abs completion counters (.then_inc on the
    instruction itself -- a separate sem_inc races with in-flight writes).
"""

import numpy as np

import concourse.bass as bass
from concourse import mybir
from concourse.bass_utils import run_bass_kernel_spmd

N_TOTAL = 33554432  # 2**25
N_CORES = 8
PER_CORE = N_TOTAL // N_CORES  # 4,194,304
P = 128  # SBUF partitions
F = 2048  # free-dim elements per tile (1 MiB DMA chunks)
BUFS = 10  # SBUF buffer slots per stream (2 * BUFS * F * 4B = 160KB/part of ~212KB)
NT = PER_CORE // (P * F)  # DRAM tiles per core

AFT = mybir.ActivationFunctionType

# Results of the most recent run (BassKernelResults), for harness introspection.
last_results = None


def _act_reciprocal(nc, out_ap, in_ap):
    """InstActivation(func=Reciprocal) without the bass-level guard.

    The guard points at accuracy concerns; measured on this hardware the
    ACT reciprocal is ~1e-6 mean / ~1e-5 max relative error over the
    label range (1e-3, 1), far inside this problem's tolerance.
    Bias/scale/alpha must be immediates for Reciprocal (same as the
    public API's Copy/Reciprocal path).
    """
    ins = [nc.scalar.lower_ap(in_ap)]
    for v in (0.0, 1.0, 0.0):  # bias, scale, alpha
        ins.append(mybir.ImmediateValue(dtype=mybir.dt.float32, value=v))
    return nc.scalar.add_instruction(
        mybir.InstActivation(
            name=nc.get_next_instruction_name(),
            func=AFT.Reciprocal,
            ins=ins,
            outs=[nc.scalar.lower_ap(out_ap)],
        )
    )


def build_nc(F=F, BUFS=BUFS, R=1):
    """Build the per-core Bass program. R = in-NEFF repetition count
    (R>1 only for benchmarking; output is identical for any R)."""
    NT = PER_CORE // (P * F)
    T = R * NT
    nc = bass.Bass()
    x_h = nc.declare_dram_parameter(
        "predictions", [NT, P, F], mybir.dt.float32, isOutput=False
    )
    y_h = nc.declare_dram_parameter(
        "labels", [NT, P, F], mybir.dt.float32, isOutput=False
    )
    out_h = nc.declare_dram_parameter(
        "partials", [P, NT], mybir.dt.float32, isOutput=True
    )

    with (
        nc.sbuf_tensor([P, BUFS * F], mybir.dt.float32) as x_sb,
        nc.sbuf_tensor([P, BUFS * F], mybir.dt.float32) as y_sb,
        nc.sbuf_tensor([P, NT], mybir.dt.float32) as acc_sb,
        nc.sbuf_tensor([P, 1], mybir.dt.float32) as neg_sb,
        nc.semaphore() as rsem,
        nc.semaphore() as msem,
        nc.semaphore() as asem,
        nc.semaphore() as bsem,
        nc.semaphore() as osem,
    ):
        sem_ctxs = [nc.semaphore(f"sem_load{s}") for s in range(BUFS)]
        sem_load = [c.__enter__() for c in sem_ctxs]
        try:
            with nc.Block() as block:
                xs = lambda s: x_sb[:, s * F : (s + 1) * F]
                ys = lambda s: y_sb[:, s * F : (s + 1) * F]

                @block.sync
                def _(sync):
                    for k in range(T):
                        i, s = k % NT, k % BUFS
                        if k >= BUFS:
                            # slot free once abs of tile k-BUFS retired
                            sync.wait_ge(asem, k - BUFS + 1)
                        sync.dma_start(out=xs(s), in_=x_h[i]).then_inc(
                            sem_load[s], 16
                        )
                        sync.dma_start(out=ys(s), in_=y_h[i]).then_inc(
                            sem_load[s], 16
                        )
                    sync.wait_ge(asem, T)
                    sync.dma_start(out=out_h[:], in_=acc_sb[:]).then_inc(osem, 16)
                    sync.wait_ge(osem, 16)

                @block.vector
                def _(vector):
                    vector.memset(neg_sb[:], -1.0).then_inc(bsem, 1)
                    for k in range(T):
                        s = k % BUFS
                        vector.wait_ge(sem_load[s], 32 * (k // BUFS + 1))
                        vector.wait_ge(rsem, k + 1)
                        nc.vector.tensor_mul(xs(s), xs(s), ys(s)).then_inc(msem, 1)

                @block.scalar
                def _(scalar):
                    scalar.wait_ge(bsem, 1)

                    def do_abs(j):
                        sj, ij = j % BUFS, j % NT
                        scalar.wait_ge(msem, j + 1)
                        nc.scalar.activation(
                            out=xs(sj),
                            in_=xs(sj),
                            func=AFT.Abs,
                            bias=neg_sb[:, 0:1],
                            scale=1.0,
                            accum_out=acc_sb[:, ij : ij + 1],
                        ).then_inc(asem, 1)

                    for k in range(T):
                        s = k % BUFS
                        scalar.wait_ge(sem_load[s], 32 * (k // BUFS + 1))
                        _act_reciprocal(nc, ys(s), ys(s)).then_inc(rsem, 1)
                        if k >= 1:
                            do_abs(k - 1)
                    if T > 0:
                        do_abs(T - 1)
        finally:
            for c in reversed(sem_ctxs):
                c.__exit__(None, None, None)
    return nc


def build_nc_dual(F=F, BUFS=BUFS, R=1, early_yload=False, tail_split=1):
    """Variant: y-tile loads issued from the ACT engine (qActDynamicHW ring)
    so x and y streams use both HWDGE rings. y-load for tile k is placed
    right after abs_{k-BUFS} in the ACT stream (abs_{k-B} implies
    mult_{k-B} retired, so the y slot is free -- no extra wait needed).

    early_yload: issue the y-load right after the msem wait but BEFORE the
    ~2us Abs instruction (same dependency -- msem>=k proves mult_{k-1}
    retired, freeing the y slot), so the DMA ring refills ~2us earlier
    per tile instead of queueing behind the Abs.

    tail_split: split the LAST tile of each pass into this many narrow
    sub-tiles. After the final DMA byte lands, the serial recip->mult->abs
    drain chain runs on a (F/tail_split)-wide tile instead of a full one,
    shrinking the single-shot tail ~tail_split-fold. Each sub-tile gets
    its own acc column (accum_out overwrites), so partials has
    NT-1+tail_split columns; the host sums all columns regardless."""
    NT = PER_CORE // (P * F)
    TS = max(1, tail_split)
    assert F % TS == 0
    # Work items per pass: NT-1 full tiles, then TS sub-tiles of the last
    # DRAM tile. (dram_tile, elem_offset, width, acc_col) per item.
    pass_items = [(i, 0, F, i) for i in range(NT - 1)]
    for c in range(TS):
        pass_items.append((NT - 1, c * (F // TS), F // TS, NT - 1 + c))
    items = pass_items * R
    T = len(items)
    ACC_COLS = NT - 1 + TS
    nc = bass.Bass()
    x_h = nc.declare_dram_parameter(
        "predictions", [NT, P, F], mybir.dt.float32, isOutput=False
    )
    y_h = nc.declare_dram_parameter(
        "labels", [NT, P, F], mybir.dt.float32, isOutput=False
    )
    out_h = nc.declare_dram_parameter(
        "partials", [P, ACC_COLS], mybir.dt.float32, isOutput=True
    )

    with (
        nc.sbuf_tensor([P, BUFS * F], mybir.dt.float32) as x_sb,
        nc.sbuf_tensor([P, BUFS * F], mybir.dt.float32) as y_sb,
        nc.sbuf_tensor([P, ACC_COLS], mybir.dt.float32) as acc_sb,
        nc.sbuf_tensor([P, 1], mybir.dt.float32) as neg_sb,
        nc.semaphore() as rsem,
        nc.semaphore() as msem,
        nc.semaphore() as asem,
        nc.semaphore() as bsem,
        nc.semaphore() as osem,
    ):
        xsem_ctxs = [nc.semaphore(f"xsem_load{s}") for s in range(BUFS)]
        ysem_ctxs = [nc.semaphore(f"ysem_load{s}") for s in range(BUFS)]
        xsem = [c.__enter__() for c in xsem_ctxs]
        ysem = [c.__enter__() for c in ysem_ctxs]
        try:
            with nc.Block() as block:
                # slot s, item width w: first w elems of the slot
                xs = lambda s, w: x_sb[:, s * F : s * F + w]
                ys = lambda s, w: y_sb[:, s * F : s * F + w]

                def dram(h, it):
                    i, off, w, _ = it
                    return h[i][:, off : off + w] if w != F else h[i]

                @block.sync
                def _(sync):
                    for k in range(T):
                        it, s = items[k], k % BUFS
                        if k >= BUFS:
                            sync.wait_ge(asem, k - BUFS + 1)
                        sync.dma_start(out=xs(s, it[2]), in_=dram(x_h, it)).then_inc(
                            xsem[s], 16
                        )
                    sync.wait_ge(asem, T)
                    sync.dma_start(out=out_h[:], in_=acc_sb[:]).then_inc(osem, 16)
                    sync.wait_ge(osem, 16)

                @block.vector
                def _(vector):
                    vector.memset(neg_sb[:], -1.0).then_inc(bsem, 1)
                    for k in range(T):
                        it, s = items[k], k % BUFS
                        vector.wait_ge(xsem[s], 16 * (k // BUFS + 1))
                        vector.wait_ge(rsem, k + 1)
                        nc.vector.tensor_mul(
                            xs(s, it[2]), xs(s, it[2]), ys(s, it[2])
                        ).then_inc(msem, 1)

                @block.scalar
                def _(scalar):
                    scalar.wait_ge(bsem, 1)

                    def y_load(k):
                        it, s = items[k], k % BUFS
                        scalar.dma_start(out=ys(s, it[2]), in_=dram(y_h, it)).then_inc(
                            ysem[s], 16
                        )

                    def do_abs(j, with_yload):
                        it, sj = items[j], j % BUFS
                        w, col = it[2], it[3]
                        scalar.wait_ge(msem, j + 1)
                        if with_yload and early_yload and j + BUFS < T:
                            # mult_j retired -> y slot j%B free; refill the
                            # ring before spending ~2us in the Abs below.
                            y_load(j + BUFS)
                        nc.scalar.activation(
                            out=xs(sj, w),
                            in_=xs(sj, w),
                            func=AFT.Abs,
                            bias=neg_sb[:, 0:1],
                            scale=1.0,
                            accum_out=acc_sb[:, col : col + 1],
                        ).then_inc(asem, 1)
                        if with_yload and not early_yload and j + BUFS < T:
                            y_load(j + BUFS)

                    for k in range(min(BUFS, T)):
                        y_load(k)
                    for k in range(T):
                        it, s = items[k], k % BUFS
                        scalar.wait_ge(ysem[s], 16 * (k // BUFS + 1))
                        _act_reciprocal(nc, ys(s, it[2]), ys(s, it[2])).then_inc(
                            rsem, 1
                        )
                        if k >= 1:
                            do_abs(k - 1, with_yload=True)
                    if T > 0:
                        do_abs(T - 1, with_yload=False)
        finally:
            for c in reversed(xsem_ctxs + ysem_ctxs):
                c.__exit__(None, None, None)
    return nc


def build_nc_dmaonly(F=F, BUFS=BUFS, R=1):
    """Timing probe: streams the same DMA traffic (x on SP ring, y on ACT
    ring) with no compute and no inter-tile waits. Output is garbage; used
    only to measure the pure DMA streaming floor."""
    NT = PER_CORE // (P * F)
    T = R * NT
    nc = bass.Bass()
    x_h = nc.declare_dram_parameter(
        "predictions", [NT, P, F], mybir.dt.float32, isOutput=False
    )
    y_h = nc.declare_dram_parameter(
        "labels", [NT, P, F], mybir.dt.float32, isOutput=False
    )
    out_h = nc.declare_dram_parameter(
        "partials", [P, NT], mybir.dt.float32, isOutput=True
    )
    with (
        nc.sbuf_tensor([P, BUFS * F], mybir.dt.float32) as x_sb,
        nc.sbuf_tensor([P, BUFS * F], mybir.dt.float32) as y_sb,
        nc.sbuf_tensor([P, NT], mybir.dt.float32) as acc_sb,
        nc.semaphore() as xsem,
        nc.semaphore() as ysem,
        nc.semaphore() as osem,
    ):
        with nc.Block() as block:
            xs = lambda s: x_sb[:, s * F : (s + 1) * F]
            ys = lambda s: y_sb[:, s * F : (s + 1) * F]

            @block.sync
            def _(sync):
                for k in range(T):
                    sync.dma_start(out=xs(k % BUFS), in_=x_h[k % NT]).then_inc(
                        xsem, 16
                    )
                sync.wait_ge(xsem, 16 * T)
                sync.wait_ge(ysem, 16 * T)
                sync.dma_start(out=out_h[:], in_=acc_sb[:]).then_inc(osem, 16)
                sync.wait_ge(osem, 16)

            @block.scalar
            def _(scalar):
                for k in range(T):
                    scalar.dma_start(out=ys(k % BUFS), in_=y_h[k % NT]).then_inc(
                        ysem, 16
                    )
    return nc


def kernel(predictions, labels):
    global last_results
    preds = np.ascontiguousarray(np.asarray(predictions, dtype=np.float32)).reshape(
        N_CORES, NT, P, F
    )
    labs = np.ascontiguousarray(np.asarray(labels, dtype=np.float32)).reshape(
        N_CORES, NT, P, F
    )
    in_maps = [{"predictions": preds[c], "labels": labs[c]} for c in range(N_CORES)]
    nc = build_nc_dual(early_yload=True, tail_split=4)
    last_results = run_bass_kernel_spmd(nc, in_maps, core_ids=list(range(N_CORES)))
    total = 0.0
    for r in last_results.results:
        total += r["partials"].astype(np.float64).sum()
    return np.float32(total / N_TOTAL * 100.0)



# revision 7
# speedup vs baseline: 1.2875x; 1.2875x over previous
"""MAPE loss on 8 Trainium2 NeuronCores (raw Bass, software-pipelined).

MAPE = mean(|pred - label| / label) * 100 over 2**25 f32 elements.

Sharding: pure data parallel. Each of the 8 cores gets a contiguous 1/8
slice of both tensors (4,194,304 elements = 16 MiB per tensor per core,
32 MiB of HBM reads per core -> memory-bound).

Default builder is build_nc_v6(y_on_sp=True) ("v6sp"). Per core, per
[128, F=2048] f32 tile (BUFS=10 slots per stream):
  SP ring:  DMA x tile and y tile into SBUF slots (both streams issued
            from the SP sequencer so neither compute engine ever stalls
            a DMA ring; x slot recycles on tile-consumed, y slot on
            recip-retired)
  ACT:      ry[k%4] <- Reciprocal(y)   (raw table act, ~1e-6 rel err,
            written to a small separate ring so the y DMA slot frees
            immediately and recip stays off the x critical path)
  DVE:      x <- x * ry                (tensor_tensor mult; z = x/y)
  abs+reduce, split 13/16 : 3/16 across engines to balance their load:
    ACT tiles: acc[:, i]      = sum_f |z - 1|   (Abs, bias=-1, accum_out)
    DVE tiles: acc[:, NT+i]   = sum_f max(z, 1) (tensor_scalar, op1=add
               acc[:, 2NT+i]  = sum_f min(z, 1)  reduces into accum_out)
               using |z-1| == max(z,1) - min(z,1); host subtracts.
Per-partition partials [128, 3*NT] are DMA'd out; the final mean is
reduced on the host in float64 (reduce_partials).

Why this shape (all numbers measured on this hardware, see ubench.py /
probe.py / compare.py):
  - Per-core HBM streaming floor drifts ~54-80us with co-tenant load
    (~420-620 GB/s/core); F=2048 (8 KiB descriptors) measured fastest.
  - Engine costs per [128,2048] tile: ACT recip 1.62us, ACT abs+accum
    1.85us, DVE mult 2.34us, DVE tensor_scalar 2.2us (the cost model's
    2x_2p fp32 mode does NOT materialize on HW; all DVE ops run ~1x).
  - So per pass: ACT recip 25.9us + 13/16 abs 24.1us ~ 50us, DVE mult
    37.4us + 3/16 TS-pairs 13.2us ~ 50us, both under the DMA floor;
    the kernel tracks the floor in either load regime.
  - abs_max is rejected by this walrus build (ISA check), hence the
    max/min tensor_scalar pair instead of a fused |.| reduce on DVE.
  - Measured vs the old build_nc_dual baseline (all abs on ACT, y DMA
    issued from the ACT stream): ~14us faster paired (~17%).

Raw Bass (not Tile): the Tile kernel-tail drain emits multi-wait CTRL
instructions this walrus build rejects ("Too many sync wait commands"),
and custom-DVE ops ("ISA wrong length") don't compile either.
Semaphore discipline:
  - xsem/ysem[s] (one per buffer slot): +16 per DMA; tile k's load is
    complete iff sem[k%B] >= 16*(k//B+1). Per-slot sems are needed
    because HWDGE completions across different tiles are not ordered.
  - rsem/msem: recip/mult completion counters; aasem/dasem: per-type
    tile-consumed counters (each type retires in order, so count-based
    slot-recycle waits stay sound even though the types race).
"""

import numpy as np

import concourse.bass as bass
from concourse import mybir
from concourse.bass_utils import run_bass_kernel_spmd

N_TOTAL = 33554432  # 2**25
N_CORES = 8
PER_CORE = N_TOTAL // N_CORES  # 4,194,304
P = 128  # SBUF partitions
F = 2048  # free-dim elements per tile (1 MiB DMA chunks)
BUFS = 10  # SBUF buffer slots per stream (2 * BUFS * F * 4B = 160KB/part of ~212KB)
NT = PER_CORE // (P * F)  # DRAM tiles per core

AFT = mybir.ActivationFunctionType

# Results of the most recent run (BassKernelResults), for harness introspection.
last_results = None


def _act_reciprocal(nc, out_ap, in_ap):
    """InstActivation(func=Reciprocal) without the bass-level guard.

    The guard points at accuracy concerns; measured on this hardware the
    ACT reciprocal is ~1e-6 mean / ~1e-5 max relative error over the
    label range (1e-3, 1), far inside this problem's tolerance.
    Bias/scale/alpha must be immediates for Reciprocal (same as the
    public API's Copy/Reciprocal path).
    """
    ins = [nc.scalar.lower_ap(in_ap)]
    for v in (0.0, 1.0, 0.0):  # bias, scale, alpha
        ins.append(mybir.ImmediateValue(dtype=mybir.dt.float32, value=v))
    return nc.scalar.add_instruction(
        mybir.InstActivation(
            name=nc.get_next_instruction_name(),
            func=AFT.Reciprocal,
            ins=ins,
            outs=[nc.scalar.lower_ap(out_ap)],
        )
    )


def build_nc(F=F, BUFS=BUFS, R=1):
    """Build the per-core Bass program. R = in-NEFF repetition count
    (R>1 only for benchmarking; output is identical for any R)."""
    NT = PER_CORE // (P * F)
    T = R * NT
    nc = bass.Bass()
    x_h = nc.declare_dram_parameter(
        "predictions", [NT, P, F], mybir.dt.float32, isOutput=False
    )
    y_h = nc.declare_dram_parameter(
        "labels", [NT, P, F], mybir.dt.float32, isOutput=False
    )
    out_h = nc.declare_dram_parameter(
        "partials", [P, NT], mybir.dt.float32, isOutput=True
    )

    with (
        nc.sbuf_tensor([P, BUFS * F], mybir.dt.float32) as x_sb,
        nc.sbuf_tensor([P, BUFS * F], mybir.dt.float32) as y_sb,
        nc.sbuf_tensor([P, NT], mybir.dt.float32) as acc_sb,
        nc.sbuf_tensor([P, 1], mybir.dt.float32) as neg_sb,
        nc.semaphore() as rsem,
        nc.semaphore() as msem,
        nc.semaphore() as asem,
        nc.semaphore() as bsem,
        nc.semaphore() as osem,
    ):
        sem_ctxs = [nc.semaphore(f"sem_load{s}") for s in range(BUFS)]
        sem_load = [c.__enter__() for c in sem_ctxs]
        try:
            with nc.Block() as block:
                xs = lambda s: x_sb[:, s * F : (s + 1) * F]
                ys = lambda s: y_sb[:, s * F : (s + 1) * F]

                @block.sync
                def _(sync):
                    for k in range(T):
                        i, s = k % NT, k % BUFS
                        if k >= BUFS:
                            # slot free once abs of tile k-BUFS retired
                            sync.wait_ge(asem, k - BUFS + 1)
                        sync.dma_start(out=xs(s), in_=x_h[i]).then_inc(
                            sem_load[s], 16
                        )
                        sync.dma_start(out=ys(s), in_=y_h[i]).then_inc(
                            sem_load[s], 16
                        )
                    sync.wait_ge(asem, T)
                    sync.dma_start(out=out_h[:], in_=acc_sb[:]).then_inc(osem, 16)
                    sync.wait_ge(osem, 16)

                @block.vector
                def _(vector):
                    vector.memset(neg_sb[:], -1.0).then_inc(bsem, 1)
                    for k in range(T):
                        s = k % BUFS
                        vector.wait_ge(sem_load[s], 32 * (k // BUFS + 1))
                        vector.wait_ge(rsem, k + 1)
                        nc.vector.tensor_mul(xs(s), xs(s), ys(s)).then_inc(msem, 1)

                @block.scalar
                def _(scalar):
                    scalar.wait_ge(bsem, 1)

                    def do_abs(j):
                        sj, ij = j % BUFS, j % NT
                        scalar.wait_ge(msem, j + 1)
                        nc.scalar.activation(
                            out=xs(sj),
                            in_=xs(sj),
                            func=AFT.Abs,
                            bias=neg_sb[:, 0:1],
                            scale=1.0,
                            accum_out=acc_sb[:, ij : ij + 1],
                        ).then_inc(asem, 1)

                    for k in range(T):
                        s = k % BUFS
                        scalar.wait_ge(sem_load[s], 32 * (k // BUFS + 1))
                        _act_reciprocal(nc, ys(s), ys(s)).then_inc(rsem, 1)
                        if k >= 1:
                            do_abs(k - 1)
                    if T > 0:
                        do_abs(T - 1)
        finally:
            for c in reversed(sem_ctxs):
                c.__exit__(None, None, None)
    return nc


def build_nc_dual(F=F, BUFS=BUFS, R=1, early_yload=False, tail_split=1):
    """Variant: y-tile loads issued from the ACT engine (qActDynamicHW ring)
    so x and y streams use both HWDGE rings. y-load for tile k is placed
    right after abs_{k-BUFS} in the ACT stream (abs_{k-B} implies
    mult_{k-B} retired, so the y slot is free -- no extra wait needed).

    early_yload: issue the y-load right after the msem wait but BEFORE the
    ~2us Abs instruction (same dependency -- msem>=k proves mult_{k-1}
    retired, freeing the y slot), so the DMA ring refills ~2us earlier
    per tile instead of queueing behind the Abs.

    tail_split: split the LAST tile of each pass into this many narrow
    sub-tiles. After the final DMA byte lands, the serial recip->mult->abs
    drain chain runs on a (F/tail_split)-wide tile instead of a full one,
    shrinking the single-shot tail ~tail_split-fold. Each sub-tile gets
    its own acc column (accum_out overwrites), so partials has
    NT-1+tail_split columns; the host sums all columns regardless."""
    NT = PER_CORE // (P * F)
    TS = max(1, tail_split)
    assert F % TS == 0
    # Work items per pass: NT-1 full tiles, then TS sub-tiles of the last
    # DRAM tile. (dram_tile, elem_offset, width, acc_col) per item.
    pass_items = [(i, 0, F, i) for i in range(NT - 1)]
    for c in range(TS):
        pass_items.append((NT - 1, c * (F // TS), F // TS, NT - 1 + c))
    items = pass_items * R
    T = len(items)
    ACC_COLS = NT - 1 + TS
    nc = bass.Bass()
    x_h = nc.declare_dram_parameter(
        "predictions", [NT, P, F], mybir.dt.float32, isOutput=False
    )
    y_h = nc.declare_dram_parameter(
        "labels", [NT, P, F], mybir.dt.float32, isOutput=False
    )
    out_h = nc.declare_dram_parameter(
        "partials", [P, ACC_COLS], mybir.dt.float32, isOutput=True
    )

    with (
        nc.sbuf_tensor([P, BUFS * F], mybir.dt.float32) as x_sb,
        nc.sbuf_tensor([P, BUFS * F], mybir.dt.float32) as y_sb,
        nc.sbuf_tensor([P, ACC_COLS], mybir.dt.float32) as acc_sb,
        nc.sbuf_tensor([P, 1], mybir.dt.float32) as neg_sb,
        nc.semaphore() as rsem,
        nc.semaphore() as msem,
        nc.semaphore() as asem,
        nc.semaphore() as bsem,
        nc.semaphore() as osem,
    ):
        xsem_ctxs = [nc.semaphore(f"xsem_load{s}") for s in range(BUFS)]
        ysem_ctxs = [nc.semaphore(f"ysem_load{s}") for s in range(BUFS)]
        xsem = [c.__enter__() for c in xsem_ctxs]
        ysem = [c.__enter__() for c in ysem_ctxs]
        try:
            with nc.Block() as block:
                # slot s, item width w: first w elems of the slot
                xs = lambda s, w: x_sb[:, s * F : s * F + w]
                ys = lambda s, w: y_sb[:, s * F : s * F + w]

                def dram(h, it):
                    i, off, w, _ = it
                    return h[i][:, off : off + w] if w != F else h[i]

                @block.sync
                def _(sync):
                    for k in range(T):
                        it, s = items[k], k % BUFS
                        if k >= BUFS:
                            sync.wait_ge(asem, k - BUFS + 1)
                        sync.dma_start(out=xs(s, it[2]), in_=dram(x_h, it)).then_inc(
                            xsem[s], 16
                        )
                    sync.wait_ge(asem, T)
                    sync.dma_start(out=out_h[:], in_=acc_sb[:]).then_inc(osem, 16)
                    sync.wait_ge(osem, 16)

                @block.vector
                def _(vector):
                    vector.memset(neg_sb[:], -1.0).then_inc(bsem, 1)
                    for k in range(T):
                        it, s = items[k], k % BUFS
                        vector.wait_ge(xsem[s], 16 * (k // BUFS + 1))
                        vector.wait_ge(rsem, k + 1)
                        nc.vector.tensor_mul(
                            xs(s, it[2]), xs(s, it[2]), ys(s, it[2])
                        ).then_inc(msem, 1)

                @block.scalar
                def _(scalar):
                    scalar.wait_ge(bsem, 1)

                    def y_load(k):
                        it, s = items[k], k % BUFS
                        scalar.dma_start(out=ys(s, it[2]), in_=dram(y_h, it)).then_inc(
                            ysem[s], 16
                        )

                    def do_abs(j, with_yload):
                        it, sj = items[j], j % BUFS
                        w, col = it[2], it[3]
                        scalar.wait_ge(msem, j + 1)
                        if with_yload and early_yload and j + BUFS < T:
                            # mult_j retired -> y slot j%B free; refill the
                            # ring before spending ~2us in the Abs below.
                            y_load(j + BUFS)
                        nc.scalar.activation(
                            out=xs(sj, w),
                            in_=xs(sj, w),
                            func=AFT.Abs,
                            bias=neg_sb[:, 0:1],
                            scale=1.0,
                            accum_out=acc_sb[:, col : col + 1],
                        ).then_inc(asem, 1)
                        if with_yload and not early_yload and j + BUFS < T:
                            y_load(j + BUFS)

                    for k in range(min(BUFS, T)):
                        y_load(k)
                    for k in range(T):
                        it, s = items[k], k % BUFS
                        scalar.wait_ge(ysem[s], 16 * (k // BUFS + 1))
                        _act_reciprocal(nc, ys(s, it[2]), ys(s, it[2])).then_inc(
                            rsem, 1
                        )
                        if k >= 1:
                            do_abs(k - 1, with_yload=True)
                    if T > 0:
                        do_abs(T - 1, with_yload=False)
        finally:
            for c in reversed(xsem_ctxs + ysem_ctxs):
                c.__exit__(None, None, None)
    return nc


def build_nc_v4(F=F, BUFS=BUFS, RB=4, R=1, y_on_sp=False):
    """v4: z-form with a separate ry ring to shorten slot-free chains.

    Measured op costs (ubench, per [128,2048] f32 tile): ACT recip 1.62us,
    ACT abs+accum 1.85us, DVE mult 2.34us. Per-pass engine busy: ACT ~66us
    (recip+abs+y-dma issue), DVE ~37us -- both under the measured ~80us
    per-core DMA streaming floor (420 GB/s), so the kernel is DMA-bound.

    vs build_nc_dual: recip writes ry into a small RB-slot ring instead of
    in-place over y, so the y DMA slot frees at recip time (same-engine
    in-order => the refill dma_start needs NO cross-engine wait), and the
    x-slot free chain is x->mult->abs (recip off the critical path).

      SP ring:  x tiles (+ y tiles too if y_on_sp)
      ACT ring: y tiles (refill issued right after the recip that frees it)
      ACT:      ry[rs] = Reciprocal(y_s); acc[:,i] = sum |z_{k-1} - 1|
      DVE:      z = x*ry (in place over x slot)
    """
    NT = PER_CORE // (P * F)
    T = R * NT
    nc = bass.Bass()
    x_h = nc.declare_dram_parameter(
        "predictions", [NT, P, F], mybir.dt.float32, isOutput=False
    )
    y_h = nc.declare_dram_parameter(
        "labels", [NT, P, F], mybir.dt.float32, isOutput=False
    )
    out_h = nc.declare_dram_parameter(
        "partials", [P, NT], mybir.dt.float32, isOutput=True
    )

    with (
        nc.sbuf_tensor([P, BUFS * F], mybir.dt.float32) as x_sb,
        nc.sbuf_tensor([P, BUFS * F], mybir.dt.float32) as y_sb,
        nc.sbuf_tensor([P, RB * F], mybir.dt.float32) as r_sb,
        nc.sbuf_tensor([P, NT], mybir.dt.float32) as acc_sb,
        nc.sbuf_tensor([P, 1], mybir.dt.float32) as neg_sb,
        nc.semaphore() as rsem,
        nc.semaphore() as msem,
        nc.semaphore() as asem,
        nc.semaphore() as bsem,
        nc.semaphore() as osem,
    ):
        xsem_ctxs = [nc.semaphore(f"xsem_load{s}") for s in range(BUFS)]
        ysem_ctxs = [nc.semaphore(f"ysem_load{s}") for s in range(BUFS)]
        xsem = [c.__enter__() for c in xsem_ctxs]
        ysem = [c.__enter__() for c in ysem_ctxs]
        try:
            with nc.Block() as block:
                xs = lambda s: x_sb[:, s * F : (s + 1) * F]
                ys = lambda s: y_sb[:, s * F : (s + 1) * F]
                rs_ = lambda r_: r_sb[:, r_ * F : (r_ + 1) * F]

                def y_load(eng, k):
                    eng.dma_start(out=ys(k % BUFS), in_=y_h[k % NT]).then_inc(
                        ysem[k % BUFS], 16
                    )

                @block.sync
                def _(sync):
                    for k in range(T):
                        s = k % BUFS
                        if k >= BUFS:
                            sync.wait_ge(asem, k - BUFS + 1)
                        sync.dma_start(out=xs(s), in_=x_h[k % NT]).then_inc(
                            xsem[s], 16
                        )
                        if y_on_sp:
                            y_load(sync, k)
                    sync.wait_ge(asem, T)
                    sync.dma_start(out=out_h[:], in_=acc_sb[:]).then_inc(osem, 16)
                    sync.wait_ge(osem, 16)

                @block.vector
                def _(vector):
                    vector.memset(neg_sb[:], -1.0).then_inc(bsem, 1)
                    for k in range(T):
                        s, r_ = k % BUFS, k % RB
                        vector.wait_ge(xsem[s], 16 * (k // BUFS + 1))
                        vector.wait_ge(rsem, k + 1)
                        nc.vector.tensor_mul(xs(s), xs(s), rs_(r_)).then_inc(
                            msem, 1
                        )

                @block.scalar
                def _(scalar):
                    scalar.wait_ge(bsem, 1)
                    if not y_on_sp:
                        for k in range(min(BUFS, T)):
                            y_load(scalar, k)

                    def do_abs(j):
                        sj, ij = j % BUFS, j % NT
                        scalar.wait_ge(msem, j + 1)
                        nc.scalar.activation(
                            out=xs(sj),
                            in_=xs(sj),
                            func=AFT.Abs,
                            bias=neg_sb[:, 0:1],
                            scale=1.0,
                            accum_out=acc_sb[:, ij : ij + 1],
                        ).then_inc(asem, 1)

                    for k in range(T):
                        s, r_ = k % BUFS, k % RB
                        scalar.wait_ge(ysem[s], 16 * (k // BUFS + 1))
                        if k >= RB:
                            # ry slot reuse: mult of k-RB must have read it
                            scalar.wait_ge(msem, k - RB + 1)
                        _act_reciprocal(nc, rs_(r_), ys(s)).then_inc(rsem, 1)
                        if not y_on_sp and k + BUFS < T:
                            # slot s was freed by the recip just issued
                            # (same engine, in order): no wait needed.
                            y_load(scalar, k + BUFS)
                        if k >= 1:
                            do_abs(k - 1)
                    if T > 0:
                        do_abs(T - 1)
        finally:
            for c in reversed(xsem_ctxs + ysem_ctxs):
                c.__exit__(None, None, None)
    return nc


def build_nc_v6(F=F, BUFS=BUFS, RB=4, R=1, y_on_sp=True, G=5):
    """v6 = v4 + absacc load-balancing across ACT and DVE.

    Every G-th tile's |z-1| sum runs on DVE as a tensor_scalar pair
    (sum max(z,1) and sum min(z,1); |z-1| == max(z,1)-min(z,1)), the rest
    on ACT (Abs bias=-1 accum). Measured per-tile costs: ACT recip 1.62us
    + abs 1.85us vs DVE mult 2.34us + TS pair 2x2.2us. G=5 balances
    ACT ~49.7us vs DVE ~51.5us per pass, so when HBM bandwidth is
    plentiful (the floor drifts 54-80us with co-tenant load) the kernel
    tracks the DMA floor instead of going ACT-bound at ~55-66us.

    Slot recycling uses per-type completion sems (aasem for ACT-abs tiles,
    dasem for DVE-abs tiles): each type retires in order, so count-based
    waits stay sound even though the two types race each other.

    Output partials [P, 3*NT]: cols i = ACT-abs sums, NT+i = DVE max sums,
    2NT+i = DVE min sums (unused cols stay 0; host adds first two groups
    and subtracts the third).
    """
    NT = PER_CORE // (P * F)
    T = R * NT
    # DVE-abs tile indices must depend only on k%NT (pass-aligned), else a
    # column would be written by different paths in different passes and the
    # host sum would double-count. ~NT/G tiles go to DVE, evenly spread.
    n_dve_tiles = max(1, round(NT / G))
    dve_set = {((2 * j + 1) * NT) // (2 * n_dve_tiles) for j in range(n_dve_tiles)}
    is_dve = lambda j: (j % NT) in dve_set
    nact = [0] * (T + 1)  # nact[j] = #ACT tiles among 0..j-1
    for j in range(T):
        nact[j + 1] = nact[j] + (0 if is_dve(j) else 1)
    ndve = lambda j: j + 1 - nact[j + 1]
    nc = bass.Bass()
    x_h = nc.declare_dram_parameter(
        "predictions", [NT, P, F], mybir.dt.float32, isOutput=False
    )
    y_h = nc.declare_dram_parameter(
        "labels", [NT, P, F], mybir.dt.float32, isOutput=False
    )
    out_h = nc.declare_dram_parameter(
        "partials", [P, 3 * NT], mybir.dt.float32, isOutput=True
    )
    AF = mybir.AluOpType

    with (
        nc.sbuf_tensor([P, BUFS * F], mybir.dt.float32) as x_sb,
        nc.sbuf_tensor([P, BUFS * F], mybir.dt.float32) as y_sb,
        nc.sbuf_tensor([P, RB * F], mybir.dt.float32) as r_sb,
        nc.sbuf_tensor([P, F], mybir.dt.float32) as dump_sb,
        nc.sbuf_tensor([P, 3 * NT], mybir.dt.float32) as acc_sb,
        nc.sbuf_tensor([P, 1], mybir.dt.float32) as neg_sb,
        nc.semaphore() as rsem,
        nc.semaphore() as msem,
        nc.semaphore() as aasem,
        nc.semaphore() as dasem,
        nc.semaphore() as bsem,
        nc.semaphore() as osem,
    ):
        xsem_ctxs = [nc.semaphore(f"xsem_load{s}") for s in range(BUFS)]
        ysem_ctxs = [nc.semaphore(f"ysem_load{s}") for s in range(BUFS)]
        xsem = [c.__enter__() for c in xsem_ctxs]
        ysem = [c.__enter__() for c in ysem_ctxs]
        try:
            with nc.Block() as block:
                xs = lambda s: x_sb[:, s * F : (s + 1) * F]
                ys = lambda s: y_sb[:, s * F : (s + 1) * F]
                rs_ = lambda r_: r_sb[:, r_ * F : (r_ + 1) * F]

                def wait_consumed(eng, j):
                    # tile j fully consumed (its x slot is free)
                    if is_dve(j):
                        eng.wait_ge(dasem, ndve(j))
                    else:
                        eng.wait_ge(aasem, nact[j + 1])

                @block.sync
                def _(sync):
                    for k in range(T):
                        s = k % BUFS
                        if k >= BUFS:
                            wait_consumed(sync, k - BUFS)
                        sync.dma_start(out=xs(s), in_=x_h[k % NT]).then_inc(
                            xsem[s], 16
                        )
                        if y_on_sp:
                            if k >= BUFS:
                                # y slot freed when its recip retired
                                sync.wait_ge(rsem, k - BUFS + 1)
                            sync.dma_start(out=ys(s), in_=y_h[k % NT]).then_inc(
                                ysem[s], 16
                            )
                    sync.wait_ge(aasem, nact[T])
                    if T - nact[T] > 0:
                        sync.wait_ge(dasem, T - nact[T])
                    sync.dma_start(out=out_h[:], in_=acc_sb[:]).then_inc(osem, 16)
                    sync.wait_ge(osem, 16)

                @block.vector
                def _(vector):
                    vector.memset(neg_sb[:], -1.0)
                    vector.memset(acc_sb[:], 0.0).then_inc(bsem, 1)
                    for k in range(T):
                        s, r_ = k % BUFS, k % RB
                        i = k % NT
                        vector.wait_ge(xsem[s], 16 * (k // BUFS + 1))
                        vector.wait_ge(rsem, k + 1)
                        nc.vector.tensor_mul(xs(s), xs(s), rs_(r_)).then_inc(
                            msem, 1
                        )
                        if is_dve(k):
                            nc.vector.tensor_scalar(
                                dump_sb[:], xs(s), 1.0, None, AF.max, AF.add,
                                accum_out=acc_sb[:, NT + i : NT + i + 1],
                            )
                            nc.vector.tensor_scalar(
                                dump_sb[:], xs(s), 1.0, None, AF.min, AF.add,
                                accum_out=acc_sb[:, 2 * NT + i : 2 * NT + i + 1],
                            ).then_inc(dasem, 1)

                @block.scalar
                def _(scalar):
                    scalar.wait_ge(bsem, 1)
                    if not y_on_sp:
                        for k in range(min(BUFS, T)):
                            scalar.dma_start(
                                out=ys(k % BUFS), in_=y_h[k % NT]
                            ).then_inc(ysem[k % BUFS], 16)

                    def do_abs(j):
                        sj, ij = j % BUFS, j % NT
                        scalar.wait_ge(msem, j + 1)
                        nc.scalar.activation(
                            out=xs(sj),
                            in_=xs(sj),
                            func=AFT.Abs,
                            bias=neg_sb[:, 0:1],
                            scale=1.0,
                            accum_out=acc_sb[:, ij : ij + 1],
                        ).then_inc(aasem, 1)

                    for k in range(T):
                        s, r_ = k % BUFS, k % RB
                        scalar.wait_ge(ysem[s], 16 * (k // BUFS + 1))
                        if k >= RB:
                            scalar.wait_ge(msem, k - RB + 1)
                        _act_reciprocal(nc, rs_(r_), ys(s)).then_inc(rsem, 1)
                        if not y_on_sp and k + BUFS < T:
                            scalar.dma_start(
                                out=ys((k + BUFS) % BUFS), in_=y_h[(k + BUFS) % NT]
                            ).then_inc(ysem[(k + BUFS) % BUFS], 16)
                        if k >= 1 and not is_dve(k - 1):
                            do_abs(k - 1)
                    if T > 0 and not is_dve(T - 1):
                        do_abs(T - 1)
        finally:
            for c in reversed(xsem_ctxs + ysem_ctxs):
                c.__exit__(None, None, None)
    return nc


def build_nc_dmaonly(F=F, BUFS=BUFS, R=1):
    """Timing probe: streams the same DMA traffic (x on SP ring, y on ACT
    ring) with no compute and no inter-tile waits. Output is garbage; used
    only to measure the pure DMA streaming floor."""
    NT = PER_CORE // (P * F)
    T = R * NT
    nc = bass.Bass()
    x_h = nc.declare_dram_parameter(
        "predictions", [NT, P, F], mybir.dt.float32, isOutput=False
    )
    y_h = nc.declare_dram_parameter(
        "labels", [NT, P, F], mybir.dt.float32, isOutput=False
    )
    out_h = nc.declare_dram_parameter(
        "partials", [P, NT], mybir.dt.float32, isOutput=True
    )
    with (
        nc.sbuf_tensor([P, BUFS * F], mybir.dt.float32) as x_sb,
        nc.sbuf_tensor([P, BUFS * F], mybir.dt.float32) as y_sb,
        nc.sbuf_tensor([P, NT], mybir.dt.float32) as acc_sb,
        nc.semaphore() as xsem,
        nc.semaphore() as ysem,
        nc.semaphore() as osem,
    ):
        with nc.Block() as block:
            xs = lambda s: x_sb[:, s * F : (s + 1) * F]
            ys = lambda s: y_sb[:, s * F : (s + 1) * F]

            @block.sync
            def _(sync):
                for k in range(T):
                    sync.dma_start(out=xs(k % BUFS), in_=x_h[k % NT]).then_inc(
                        xsem, 16
                    )
                sync.wait_ge(xsem, 16 * T)
                sync.wait_ge(ysem, 16 * T)
                sync.dma_start(out=out_h[:], in_=acc_sb[:]).then_inc(osem, 16)
                sync.wait_ge(osem, 16)

            @block.scalar
            def _(scalar):
                for k in range(T):
                    scalar.dma_start(out=ys(k % BUFS), in_=y_h[k % NT]).then_inc(
                        ysem, 16
                    )
    return nc


# Selected variant for kernel() and the bench. "v6sp": x+y DMA issue on the
# SP ring, ACT recip (+13/16 of abs+accum), DVE mult (+3/16 of the abs work
# as max/min tensor_scalar pairs). See build_nc_v6.
VARIANT = "v6sp"


def build_kernel_nc(R=1):
    if VARIANT == "v6sp":
        return build_nc_v6(R=R, y_on_sp=True)
    if VARIANT == "v6":
        return build_nc_v6(R=R, y_on_sp=False)
    if VARIANT == "v4":
        return build_nc_v4(R=R)
    if VARIANT == "v4sp":
        return build_nc_v4(R=R, y_on_sp=True)
    return build_nc_dual(early_yload=True, tail_split=4, R=R)


def reduce_partials(partials):
    """Host-side reduction of one core's partials to a float64 sum."""
    p = partials.astype(np.float64)
    if VARIANT.startswith("v6"):
        NTC = p.shape[-1] // 3
        return p[..., :NTC].sum() + p[..., NTC : 2 * NTC].sum() - p[..., 2 * NTC :].sum()
    return p.sum()


def kernel(predictions, labels):
    global last_results
    preds = np.ascontiguousarray(np.asarray(predictions, dtype=np.float32)).reshape(
        N_CORES, NT, P, F
    )
    labs = np.ascontiguousarray(np.asarray(labels, dtype=np.float32)).reshape(
        N_CORES, NT, P, F
    )
    in_maps = [{"predictions": preds[c], "labels": labs[c]} for c in range(N_CORES)]
    nc = build_kernel_nc()
    last_results = run_bass_kernel_spmd(nc, in_maps, core_ids=list(range(N_CORES)))
    total = 0.0
    for r in last_results.results:
        total += reduce_partials(r["partials"])
    return np.float32(total / N_TOTAL * 100.0)



# revision 16
# speedup vs baseline: 1.8776x; 1.4583x over previous
"""MAPE loss on 8 Trainium2 NeuronCores (raw Bass, software-pipelined).

MAPE = mean(|pred - label| / label) * 100 over 2**25 f32 elements.

Sharding: pure data parallel. Each of the 8 cores gets a contiguous 1/8
slice of both tensors (4,194,304 elements = 16 MiB per tensor per core,
32 MiB of HBM reads per core -> memory-bound).

Default builder is build_nc_v6(y_on_sp=True) ("v6sp"). Per core, per
[128, F=2048] f32 tile (BUFS=10 slots per stream):
  SP ring:  DMA x tile and y tile into SBUF slots (both streams issued
            from the SP sequencer so neither compute engine ever stalls
            a DMA ring; x slot recycles on tile-consumed, y slot on
            recip-retired)
  ACT:      ry[k%4] <- Reciprocal(y)   (raw table act, ~1e-6 rel err,
            written to a small separate ring so the y DMA slot frees
            immediately and recip stays off the x critical path)
  DVE:      x <- x * ry                (tensor_tensor mult; z = x/y)
  abs+reduce, split 13/16 : 3/16 across engines to balance their load:
    ACT tiles: acc[:, i]      = sum_f |z - 1|   (Abs, bias=-1, accum_out)
    DVE tiles: acc[:, NT+i]   = sum_f max(z, 1) (tensor_scalar, op1=add
               acc[:, 2NT+i]  = sum_f min(z, 1)  reduces into accum_out)
               using |z-1| == max(z,1) - min(z,1); host subtracts.
Per-partition partials [128, 3*NT] are DMA'd out; the final mean is
reduced on the host in float64 (reduce_partials).

Why this shape (all numbers measured on this hardware, see ubench.py /
probe.py / compare.py):
  - Per-core HBM streaming floor drifts ~54-80us with co-tenant load
    (~420-620 GB/s/core); F=2048 (8 KiB descriptors) measured fastest.
  - Engine costs per [128,2048] tile: ACT recip 1.62us, ACT abs+accum
    1.85us, DVE mult 2.34us, DVE tensor_scalar 2.2us (the cost model's
    2x_2p fp32 mode does NOT materialize on HW; all DVE ops run ~1x).
  - So per pass: ACT recip 25.9us + 13/16 abs 24.1us ~ 50us, DVE mult
    37.4us + 3/16 TS-pairs 13.2us ~ 50us, both under the DMA floor;
    the kernel tracks the floor in either load regime.
  - abs_max is rejected by this walrus build (ISA check), hence the
    max/min tensor_scalar pair instead of a fused |.| reduce on DVE.
  - Measured vs the old build_nc_dual baseline (all abs on ACT, y DMA
    issued from the ACT stream): ~14us faster paired (~17%).

Raw Bass (not Tile): the Tile kernel-tail drain emits multi-wait CTRL
instructions this walrus build rejects ("Too many sync wait commands"),
and custom-DVE ops ("ISA wrong length") don't compile either.
Semaphore discipline:
  - xsem/ysem[s] (one per buffer slot): +16 per DMA; tile k's load is
    complete iff sem[k%B] >= 16*(k//B+1). Per-slot sems are needed
    because HWDGE completions across different tiles are not ordered.
  - rsem/msem: recip/mult completion counters; aasem/dasem: per-type
    tile-consumed counters (each type retires in order, so count-based
    slot-recycle waits stay sound even though the types race).
"""

import numpy as np

import concourse.bass as bass
from concourse import mybir
from concourse.bass_utils import run_bass_kernel_spmd

N_TOTAL = 33554432  # 2**25
N_CORES = 8
PER_CORE = N_TOTAL // N_CORES  # 4,194,304
P = 128  # SBUF partitions
F = 2048  # free-dim elements per tile (1 MiB DMA chunks)
BUFS = 10  # SBUF buffer slots per stream (2 * BUFS * F * 4B = 160KB/part of ~212KB)
NT = PER_CORE // (P * F)  # DRAM tiles per core

AFT = mybir.ActivationFunctionType

# Results of the most recent run (BassKernelResults), for harness introspection.
last_results = None


def _act_reciprocal(nc, out_ap, in_ap):
    """InstActivation(func=Reciprocal) without the bass-level guard.

    The guard points at accuracy concerns; measured on this hardware the
    ACT reciprocal is ~1e-6 mean / ~1e-5 max relative error over the
    label range (1e-3, 1), far inside this problem's tolerance.
    Bias/scale/alpha must be immediates for Reciprocal (same as the
    public API's Copy/Reciprocal path).
    """
    ins = [nc.scalar.lower_ap(in_ap)]
    for v in (0.0, 1.0, 0.0):  # bias, scale, alpha
        ins.append(mybir.ImmediateValue(dtype=mybir.dt.float32, value=v))
    return nc.scalar.add_instruction(
        mybir.InstActivation(
            name=nc.get_next_instruction_name(),
            func=AFT.Reciprocal,
            ins=ins,
            outs=[nc.scalar.lower_ap(out_ap)],
        )
    )


def build_nc(F=F, BUFS=BUFS, R=1):
    """Build the per-core Bass program. R = in-NEFF repetition count
    (R>1 only for benchmarking; output is identical for any R)."""
    NT = PER_CORE // (P * F)
    T = R * NT
    nc = bass.Bass()
    x_h = nc.declare_dram_parameter(
        "predictions", [NT, P, F], mybir.dt.float32, isOutput=False
    )
    y_h = nc.declare_dram_parameter(
        "labels", [NT, P, F], mybir.dt.float32, isOutput=False
    )
    out_h = nc.declare_dram_parameter(
        "partials", [P, NT], mybir.dt.float32, isOutput=True
    )

    with (
        nc.sbuf_tensor([P, BUFS * F], mybir.dt.float32) as x_sb,
        nc.sbuf_tensor([P, BUFS * F], mybir.dt.float32) as y_sb,
        nc.sbuf_tensor([P, NT], mybir.dt.float32) as acc_sb,
        nc.sbuf_tensor([P, 1], mybir.dt.float32) as neg_sb,
        nc.semaphore() as rsem,
        nc.semaphore() as msem,
        nc.semaphore() as asem,
        nc.semaphore() as bsem,
        nc.semaphore() as osem,
    ):
        sem_ctxs = [nc.semaphore(f"sem_load{s}") for s in range(BUFS)]
        sem_load = [c.__enter__() for c in sem_ctxs]
        try:
            with nc.Block() as block:
                xs = lambda s: x_sb[:, s * F : (s + 1) * F]
                ys = lambda s: y_sb[:, s * F : (s + 1) * F]

                @block.sync
                def _(sync):
                    for k in range(T):
                        i, s = k % NT, k % BUFS
                        if k >= BUFS:
                            # slot free once abs of tile k-BUFS retired
                            sync.wait_ge(asem, k - BUFS + 1)
                        sync.dma_start(out=xs(s), in_=x_h[i]).then_inc(
                            sem_load[s], 16
                        )
                        sync.dma_start(out=ys(s), in_=y_h[i]).then_inc(
                            sem_load[s], 16
                        )
                    sync.wait_ge(asem, T)
                    sync.dma_start(out=out_h[:], in_=acc_sb[:]).then_inc(osem, 16)
                    sync.wait_ge(osem, 16)

                @block.vector
                def _(vector):
                    vector.memset(neg_sb[:], -1.0).then_inc(bsem, 1)
                    for k in range(T):
                        s = k % BUFS
                        vector.wait_ge(sem_load[s], 32 * (k // BUFS + 1))
                        vector.wait_ge(rsem, k + 1)
                        nc.vector.tensor_mul(xs(s), xs(s), ys(s)).then_inc(msem, 1)

                @block.scalar
                def _(scalar):
                    scalar.wait_ge(bsem, 1)

                    def do_abs(j):
                        sj, ij = j % BUFS, j % NT
                        scalar.wait_ge(msem, j + 1)
                        nc.scalar.activation(
                            out=xs(sj),
                            in_=xs(sj),
                            func=AFT.Abs,
                            bias=neg_sb[:, 0:1],
                            scale=1.0,
                            accum_out=acc_sb[:, ij : ij + 1],
                        ).then_inc(asem, 1)

                    for k in range(T):
                        s = k % BUFS
                        scalar.wait_ge(sem_load[s], 32 * (k // BUFS + 1))
                        _act_reciprocal(nc, ys(s), ys(s)).then_inc(rsem, 1)
                        if k >= 1:
                            do_abs(k - 1)
                    if T > 0:
                        do_abs(T - 1)
        finally:
            for c in reversed(sem_ctxs):
                c.__exit__(None, None, None)
    return nc


def build_nc_dual(F=F, BUFS=BUFS, R=1, early_yload=False, tail_split=1):
    """Variant: y-tile loads issued from the ACT engine (qActDynamicHW ring)
    so x and y streams use both HWDGE rings. y-load for tile k is placed
    right after abs_{k-BUFS} in the ACT stream (abs_{k-B} implies
    mult_{k-B} retired, so the y slot is free -- no extra wait needed).

    early_yload: issue the y-load right after the msem wait but BEFORE the
    ~2us Abs instruction (same dependency -- msem>=k proves mult_{k-1}
    retired, freeing the y slot), so the DMA ring refills ~2us earlier
    per tile instead of queueing behind the Abs.

    tail_split: split the LAST tile of each pass into this many narrow
    sub-tiles. After the final DMA byte lands, the serial recip->mult->abs
    drain chain runs on a (F/tail_split)-wide tile instead of a full one,
    shrinking the single-shot tail ~tail_split-fold. Each sub-tile gets
    its own acc column (accum_out overwrites), so partials has
    NT-1+tail_split columns; the host sums all columns regardless."""
    NT = PER_CORE // (P * F)
    TS = max(1, tail_split)
    assert F % TS == 0
    # Work items per pass: NT-1 full tiles, then TS sub-tiles of the last
    # DRAM tile. (dram_tile, elem_offset, width, acc_col) per item.
    pass_items = [(i, 0, F, i) for i in range(NT - 1)]
    for c in range(TS):
        pass_items.append((NT - 1, c * (F // TS), F // TS, NT - 1 + c))
    items = pass_items * R
    T = len(items)
    ACC_COLS = NT - 1 + TS
    nc = bass.Bass()
    x_h = nc.declare_dram_parameter(
        "predictions", [NT, P, F], mybir.dt.float32, isOutput=False
    )
    y_h = nc.declare_dram_parameter(
        "labels", [NT, P, F], mybir.dt.float32, isOutput=False
    )
    out_h = nc.declare_dram_parameter(
        "partials", [P, ACC_COLS], mybir.dt.float32, isOutput=True
    )

    with (
        nc.sbuf_tensor([P, BUFS * F], mybir.dt.float32) as x_sb,
        nc.sbuf_tensor([P, BUFS * F], mybir.dt.float32) as y_sb,
        nc.sbuf_tensor([P, ACC_COLS], mybir.dt.float32) as acc_sb,
        nc.sbuf_tensor([P, 1], mybir.dt.float32) as neg_sb,
        nc.semaphore() as rsem,
        nc.semaphore() as msem,
        nc.semaphore() as asem,
        nc.semaphore() as bsem,
        nc.semaphore() as osem,
    ):
        xsem_ctxs = [nc.semaphore(f"xsem_load{s}") for s in range(BUFS)]
        ysem_ctxs = [nc.semaphore(f"ysem_load{s}") for s in range(BUFS)]
        xsem = [c.__enter__() for c in xsem_ctxs]
        ysem = [c.__enter__() for c in ysem_ctxs]
        try:
            with nc.Block() as block:
                # slot s, item width w: first w elems of the slot
                xs = lambda s, w: x_sb[:, s * F : s * F + w]
                ys = lambda s, w: y_sb[:, s * F : s * F + w]

                def dram(h, it):
                    i, off, w, _ = it
                    return h[i][:, off : off + w] if w != F else h[i]

                @block.sync
                def _(sync):
                    for k in range(T):
                        it, s = items[k], k % BUFS
                        if k >= BUFS:
                            sync.wait_ge(asem, k - BUFS + 1)
                        sync.dma_start(out=xs(s, it[2]), in_=dram(x_h, it)).then_inc(
                            xsem[s], 16
                        )
                    sync.wait_ge(asem, T)
                    sync.dma_start(out=out_h[:], in_=acc_sb[:]).then_inc(osem, 16)
                    sync.wait_ge(osem, 16)

                @block.vector
                def _(vector):
                    vector.memset(neg_sb[:], -1.0).then_inc(bsem, 1)
                    for k in range(T):
                        it, s = items[k], k % BUFS
                        vector.wait_ge(xsem[s], 16 * (k // BUFS + 1))
                        vector.wait_ge(rsem, k + 1)
                        nc.vector.tensor_mul(
                            xs(s, it[2]), xs(s, it[2]), ys(s, it[2])
                        ).then_inc(msem, 1)

                @block.scalar
                def _(scalar):
                    scalar.wait_ge(bsem, 1)

                    def y_load(k):
                        it, s = items[k], k % BUFS
                        scalar.dma_start(out=ys(s, it[2]), in_=dram(y_h, it)).then_inc(
                            ysem[s], 16
                        )

                    def do_abs(j, with_yload):
                        it, sj = items[j], j % BUFS
                        w, col = it[2], it[3]
                        scalar.wait_ge(msem, j + 1)
                        if with_yload and early_yload and j + BUFS < T:
                            # mult_j retired -> y slot j%B free; refill the
                            # ring before spending ~2us in the Abs below.
                            y_load(j + BUFS)
                        nc.scalar.activation(
                            out=xs(sj, w),
                            in_=xs(sj, w),
                            func=AFT.Abs,
                            bias=neg_sb[:, 0:1],
                            scale=1.0,
                            accum_out=acc_sb[:, col : col + 1],
                        ).then_inc(asem, 1)
                        if with_yload and not early_yload and j + BUFS < T:
                            y_load(j + BUFS)

                    for k in range(min(BUFS, T)):
                        y_load(k)
                    for k in range(T):
                        it, s = items[k], k % BUFS
                        scalar.wait_ge(ysem[s], 16 * (k // BUFS + 1))
                        _act_reciprocal(nc, ys(s, it[2]), ys(s, it[2])).then_inc(
                            rsem, 1
                        )
                        if k >= 1:
                            do_abs(k - 1, with_yload=True)
                    if T > 0:
                        do_abs(T - 1, with_yload=False)
        finally:
            for c in reversed(xsem_ctxs + ysem_ctxs):
                c.__exit__(None, None, None)
    return nc


def build_nc_v4(F=F, BUFS=BUFS, RB=4, R=1, y_on_sp=False):
    """v4: z-form with a separate ry ring to shorten slot-free chains.

    Measured op costs (ubench, per [128,2048] f32 tile): ACT recip 1.62us,
    ACT abs+accum 1.85us, DVE mult 2.34us. Per-pass engine busy: ACT ~66us
    (recip+abs+y-dma issue), DVE ~37us -- both under the measured ~80us
    per-core DMA streaming floor (420 GB/s), so the kernel is DMA-bound.

    vs build_nc_dual: recip writes ry into a small RB-slot ring instead of
    in-place over y, so the y DMA slot frees at recip time (same-engine
    in-order => the refill dma_start needs NO cross-engine wait), and the
    x-slot free chain is x->mult->abs (recip off the critical path).

      SP ring:  x tiles (+ y tiles too if y_on_sp)
      ACT ring: y tiles (refill issued right after the recip that frees it)
      ACT:      ry[rs] = Reciprocal(y_s); acc[:,i] = sum |z_{k-1} - 1|
      DVE:      z = x*ry (in place over x slot)
    """
    NT = PER_CORE // (P * F)
    T = R * NT
    nc = bass.Bass()
    x_h = nc.declare_dram_parameter(
        "predictions", [NT, P, F], mybir.dt.float32, isOutput=False
    )
    y_h = nc.declare_dram_parameter(
        "labels", [NT, P, F], mybir.dt.float32, isOutput=False
    )
    out_h = nc.declare_dram_parameter(
        "partials", [P, NT], mybir.dt.float32, isOutput=True
    )

    with (
        nc.sbuf_tensor([P, BUFS * F], mybir.dt.float32) as x_sb,
        nc.sbuf_tensor([P, BUFS * F], mybir.dt.float32) as y_sb,
        nc.sbuf_tensor([P, RB * F], mybir.dt.float32) as r_sb,
        nc.sbuf_tensor([P, NT], mybir.dt.float32) as acc_sb,
        nc.sbuf_tensor([P, 1], mybir.dt.float32) as neg_sb,
        nc.semaphore() as rsem,
        nc.semaphore() as msem,
        nc.semaphore() as asem,
        nc.semaphore() as bsem,
        nc.semaphore() as osem,
    ):
        xsem_ctxs = [nc.semaphore(f"xsem_load{s}") for s in range(BUFS)]
        ysem_ctxs = [nc.semaphore(f"ysem_load{s}") for s in range(BUFS)]
        xsem = [c.__enter__() for c in xsem_ctxs]
        ysem = [c.__enter__() for c in ysem_ctxs]
        try:
            with nc.Block() as block:
                xs = lambda s: x_sb[:, s * F : (s + 1) * F]
                ys = lambda s: y_sb[:, s * F : (s + 1) * F]
                rs_ = lambda r_: r_sb[:, r_ * F : (r_ + 1) * F]

                def y_load(eng, k):
                    eng.dma_start(out=ys(k % BUFS), in_=y_h[k % NT]).then_inc(
                        ysem[k % BUFS], 16
                    )

                @block.sync
                def _(sync):
                    for k in range(T):
                        s = k % BUFS
                        if k >= BUFS:
                            sync.wait_ge(asem, k - BUFS + 1)
                        sync.dma_start(out=xs(s), in_=x_h[k % NT]).then_inc(
                            xsem[s], 16
                        )
                        if y_on_sp:
                            y_load(sync, k)
                    sync.wait_ge(asem, T)
                    sync.dma_start(out=out_h[:], in_=acc_sb[:]).then_inc(osem, 16)
                    sync.wait_ge(osem, 16)

                @block.vector
                def _(vector):
                    vector.memset(neg_sb[:], -1.0).then_inc(bsem, 1)
                    for k in range(T):
                        s, r_ = k % BUFS, k % RB
                        vector.wait_ge(xsem[s], 16 * (k // BUFS + 1))
                        vector.wait_ge(rsem, k + 1)
                        nc.vector.tensor_mul(xs(s), xs(s), rs_(r_)).then_inc(
                            msem, 1
                        )

                @block.scalar
                def _(scalar):
                    scalar.wait_ge(bsem, 1)
                    if not y_on_sp:
                        for k in range(min(BUFS, T)):
                            y_load(scalar, k)

                    def do_abs(j):
                        sj, ij = j % BUFS, j % NT
                        scalar.wait_ge(msem, j + 1)
                        nc.scalar.activation(
                            out=xs(sj),
                            in_=xs(sj),
                            func=AFT.Abs,
                            bias=neg_sb[:, 0:1],
                            scale=1.0,
                            accum_out=acc_sb[:, ij : ij + 1],
                        ).then_inc(asem, 1)

                    for k in range(T):
                        s, r_ = k % BUFS, k % RB
                        scalar.wait_ge(ysem[s], 16 * (k // BUFS + 1))
                        if k >= RB:
                            # ry slot reuse: mult of k-RB must have read it
                            scalar.wait_ge(msem, k - RB + 1)
                        _act_reciprocal(nc, rs_(r_), ys(s)).then_inc(rsem, 1)
                        if not y_on_sp and k + BUFS < T:
                            # slot s was freed by the recip just issued
                            # (same engine, in order): no wait needed.
                            y_load(scalar, k + BUFS)
                        if k >= 1:
                            do_abs(k - 1)
                    if T > 0:
                        do_abs(T - 1)
        finally:
            for c in reversed(xsem_ctxs + ysem_ctxs):
                c.__exit__(None, None, None)
    return nc


def build_nc_v6(F=F, BUFS=BUFS, RB=4, R=1, y_on_sp=True, G=5, z16=False):
    """v6 = v4 + absacc load-balancing across ACT and DVE.

    Every G-th tile's |z-1| sum runs on DVE as a tensor_scalar pair
    (sum max(z,1) and sum min(z,1); |z-1| == max(z,1)-min(z,1)), the rest
    on ACT (Abs bias=-1 accum). Measured per-tile costs: ACT recip 1.62us
    + abs 1.85us vs DVE mult 2.34us + TS pair 2x2.2us. G=5 balances
    ACT ~49.7us vs DVE ~51.5us per pass, so when HBM bandwidth is
    plentiful (the floor drifts 54-80us with co-tenant load) the kernel
    tracks the DMA floor instead of going ACT-bound at ~55-66us.

    Slot recycling uses per-type completion sems (aasem for ACT-abs tiles,
    dasem for DVE-abs tiles): each type retires in order, so count-based
    waits stay sound even though the two types race each other.

    Output partials [P, 3*NT]: cols i = ACT-abs sums, NT+i = DVE max sums,
    2NT+i = DVE min sums (unused cols stay 0; host adds first two groups
    and subtracts the third).
    """
    NT = PER_CORE // (P * F)
    T = R * NT
    # DVE-abs tile indices must depend only on k%NT (pass-aligned), else a
    # column would be written by different paths in different passes and the
    # host sum would double-count. ~NT/G tiles go to DVE, evenly spread.
    n_dve_tiles = max(1, round(NT / G))
    dve_set = {((2 * j + 1) * NT) // (2 * n_dve_tiles) for j in range(n_dve_tiles)}
    is_dve = lambda j: (j % NT) in dve_set
    nact = [0] * (T + 1)  # nact[j] = #ACT tiles among 0..j-1
    for j in range(T):
        nact[j + 1] = nact[j] + (0 if is_dve(j) else 1)
    ndve = lambda j: j + 1 - nact[j + 1]
    nc = bass.Bass()
    x_h = nc.declare_dram_parameter(
        "predictions", [NT, P, F], mybir.dt.float32, isOutput=False
    )
    y_h = nc.declare_dram_parameter(
        "labels", [NT, P, F], mybir.dt.float32, isOutput=False
    )
    out_h = nc.declare_dram_parameter(
        "partials", [P, 3 * NT], mybir.dt.float32, isOutput=True
    )
    AF = mybir.AluOpType

    # z16: the mult writes z as bf16 into the FIRST half of its x slot
    # (write offset 2e trails the 4e read offset, so no unread input is
    # clobbered); abs/TS read the bf16 z and dump their elementwise output
    # into the slot's second half. Halves z read/write SBUF traffic and
    # frees the dump buffer, paying ~0.2% per-element rounding noise that
    # averages out in the 2**25-element mean.
    dump_cols = 0 if z16 else F
    with (
        nc.sbuf_tensor([P, BUFS * F], mybir.dt.float32) as x_sb,
        nc.sbuf_tensor([P, BUFS * F], mybir.dt.float32) as y_sb,
        nc.sbuf_tensor([P, RB * F], mybir.dt.float32) as r_sb,
        nc.sbuf_tensor([P, max(1, dump_cols)], mybir.dt.float32) as dump_sb,
        nc.sbuf_tensor([P, 3 * NT], mybir.dt.float32) as acc_sb,
        nc.sbuf_tensor([P, 1], mybir.dt.float32) as neg_sb,
        nc.semaphore() as rsem,
        nc.semaphore() as msem,
        nc.semaphore() as aasem,
        nc.semaphore() as dasem,
        nc.semaphore() as bsem,
        nc.semaphore() as osem,
    ):
        xsem_ctxs = [nc.semaphore(f"xsem_load{s}") for s in range(BUFS)]
        ysem_ctxs = [nc.semaphore(f"ysem_load{s}") for s in range(BUFS)]
        xsem = [c.__enter__() for c in xsem_ctxs]
        ysem = [c.__enter__() for c in ysem_ctxs]
        try:
            with nc.Block() as block:
                xs = lambda s: x_sb[:, s * F : (s + 1) * F]
                ys = lambda s: y_sb[:, s * F : (s + 1) * F]
                rs_ = lambda r_: r_sb[:, r_ * F : (r_ + 1) * F]
                if z16:
                    xb = lambda s: xs(s).bitcast(mybir.dt.bfloat16)
                    zs = lambda s: xb(s)[:, 0:F]  # bf16 z, first half
                    os_ = lambda s: xb(s)[:, F : 2 * F]  # scratch, 2nd half
                else:
                    zs = xs
                    os_ = lambda s: dump_sb[:]

                def wait_consumed(eng, j):
                    # tile j fully consumed (its x slot is free)
                    if is_dve(j):
                        eng.wait_ge(dasem, ndve(j))
                    else:
                        eng.wait_ge(aasem, nact[j + 1])

                @block.sync
                def _(sync):
                    for k in range(T):
                        s = k % BUFS
                        if k >= BUFS:
                            wait_consumed(sync, k - BUFS)
                        sync.dma_start(out=xs(s), in_=x_h[k % NT]).then_inc(
                            xsem[s], 16
                        )
                        if y_on_sp:
                            if k >= BUFS:
                                # y slot freed when its recip retired
                                sync.wait_ge(rsem, k - BUFS + 1)
                            sync.dma_start(out=ys(s), in_=y_h[k % NT]).then_inc(
                                ysem[s], 16
                            )
                    sync.wait_ge(aasem, nact[T])
                    if T - nact[T] > 0:
                        sync.wait_ge(dasem, T - nact[T])
                    sync.dma_start(out=out_h[:], in_=acc_sb[:]).then_inc(osem, 16)
                    sync.wait_ge(osem, 16)

                @block.vector
                def _(vector):
                    vector.memset(neg_sb[:], -1.0)
                    vector.memset(acc_sb[:], 0.0).then_inc(bsem, 1)
                    for k in range(T):
                        s, r_ = k % BUFS, k % RB
                        i = k % NT
                        vector.wait_ge(xsem[s], 16 * (k // BUFS + 1))
                        vector.wait_ge(rsem, k + 1)
                        nc.vector.tensor_mul(zs(s), xs(s), rs_(r_)).then_inc(
                            msem, 1
                        )
                        if is_dve(k):
                            nc.vector.tensor_scalar(
                                os_(s), zs(s), 1.0, None, AF.max, AF.add,
                                accum_out=acc_sb[:, NT + i : NT + i + 1],
                            )
                            nc.vector.tensor_scalar(
                                os_(s), zs(s), 1.0, None, AF.min, AF.add,
                                accum_out=acc_sb[:, 2 * NT + i : 2 * NT + i + 1],
                            ).then_inc(dasem, 1)

                @block.scalar
                def _(scalar):
                    scalar.wait_ge(bsem, 1)
                    if not y_on_sp:
                        for k in range(min(BUFS, T)):
                            scalar.dma_start(
                                out=ys(k % BUFS), in_=y_h[k % NT]
                            ).then_inc(ysem[k % BUFS], 16)

                    def do_abs(j):
                        sj, ij = j % BUFS, j % NT
                        scalar.wait_ge(msem, j + 1)
                        nc.scalar.activation(
                            out=os_(sj) if z16 else xs(sj),
                            in_=zs(sj),
                            func=AFT.Abs,
                            bias=neg_sb[:, 0:1],
                            scale=1.0,
                            accum_out=acc_sb[:, ij : ij + 1],
                        ).then_inc(aasem, 1)

                    for k in range(T):
                        s, r_ = k % BUFS, k % RB
                        scalar.wait_ge(ysem[s], 16 * (k // BUFS + 1))
                        if k >= RB:
                            scalar.wait_ge(msem, k - RB + 1)
                        _act_reciprocal(nc, rs_(r_), ys(s)).then_inc(rsem, 1)
                        if not y_on_sp and k + BUFS < T:
                            scalar.dma_start(
                                out=ys((k + BUFS) % BUFS), in_=y_h[(k + BUFS) % NT]
                            ).then_inc(ysem[(k + BUFS) % BUFS], 16)
                        if k >= 1 and not is_dve(k - 1):
                            do_abs(k - 1)
                    if T > 0 and not is_dve(T - 1):
                        do_abs(T - 1)
        finally:
            for c in reversed(xsem_ctxs + ysem_ctxs):
                c.__exit__(None, None, None)
    return nc


def build_nc_dmaonly(F=F, BUFS=BUFS, R=1):
    """Timing probe: streams the same DMA traffic (x on SP ring, y on ACT
    ring) with no compute and no inter-tile waits. Output is garbage; used
    only to measure the pure DMA streaming floor."""
    NT = PER_CORE // (P * F)
    T = R * NT
    nc = bass.Bass()
    x_h = nc.declare_dram_parameter(
        "predictions", [NT, P, F], mybir.dt.float32, isOutput=False
    )
    y_h = nc.declare_dram_parameter(
        "labels", [NT, P, F], mybir.dt.float32, isOutput=False
    )
    out_h = nc.declare_dram_parameter(
        "partials", [P, NT], mybir.dt.float32, isOutput=True
    )
    with (
        nc.sbuf_tensor([P, BUFS * F], mybir.dt.float32) as x_sb,
        nc.sbuf_tensor([P, BUFS * F], mybir.dt.float32) as y_sb,
        nc.sbuf_tensor([P, NT], mybir.dt.float32) as acc_sb,
        nc.semaphore() as xsem,
        nc.semaphore() as ysem,
        nc.semaphore() as osem,
    ):
        with nc.Block() as block:
            xs = lambda s: x_sb[:, s * F : (s + 1) * F]
            ys = lambda s: y_sb[:, s * F : (s + 1) * F]

            @block.sync
            def _(sync):
                for k in range(T):
                    sync.dma_start(out=xs(k % BUFS), in_=x_h[k % NT]).then_inc(
                        xsem, 16
                    )
                sync.wait_ge(xsem, 16 * T)
                sync.wait_ge(ysem, 16 * T)
                sync.dma_start(out=out_h[:], in_=acc_sb[:]).then_inc(osem, 16)
                sync.wait_ge(osem, 16)

            @block.scalar
            def _(scalar):
                for k in range(T):
                    scalar.dma_start(out=ys(k % BUFS), in_=y_h[k % NT]).then_inc(
                        ysem, 16
                    )
    return nc


def build_nc_v6ts(F=F, BUFS=BUFS, RB=4, R=1, G=5, TS=4):
    """v6sp with the LAST tile of each pass split into TS sub-tiles.

    Single-shot latency: after the final DMA byte lands, the serial
    recip->mult->abs drain runs on an (F/TS)-wide tile instead of a full
    one, shrinking the tail from ~8us to ~3us. Used for the R=1 NEFF that
    kernel() executes; the R>1 bench NEFFs keep the unsplit schedule
    (the sub-tiles would only add per-op overhead to the steady state).

    Structure, sems and the ACT/DVE abs split are exactly build_nc_v6
    (y_on_sp=True), just iterated over an items list with per-item widths.
    Output partials [P, 3*(NT-1+TS)], same 3-group layout.
    """
    NT = PER_CORE // (P * F)
    n_dve_tiles = max(1, round(NT / G))
    dve_set = {((2 * j + 1) * NT) // (2 * n_dve_tiles) for j in range(n_dve_tiles)}
    dve_set.discard(NT - 1)  # last tile is split; its sub-tiles stay on ACT
    # item = (dram_tile, elem_offset, width, acc_col, on_dve)
    pass_items = [(i, 0, F, i, i in dve_set) for i in range(NT - 1)]
    W = F // TS
    for j in range(TS):
        pass_items.append((NT - 1, j * W, W, NT - 1 + j, False))
    items = pass_items * R
    T = len(items)
    ACC = NT - 1 + TS
    is_dve = lambda j: items[j][4]
    nact = [0] * (T + 1)
    for j in range(T):
        nact[j + 1] = nact[j] + (0 if is_dve(j) else 1)
    ndve = lambda j: j + 1 - nact[j + 1]
    nc = bass.Bass()
    x_h = nc.declare_dram_parameter(
        "predictions", [NT, P, F], mybir.dt.float32, isOutput=False
    )
    y_h = nc.declare_dram_parameter(
        "labels", [NT, P, F], mybir.dt.float32, isOutput=False
    )
    out_h = nc.declare_dram_parameter(
        "partials", [P, 3 * ACC], mybir.dt.float32, isOutput=True
    )
    AF = mybir.AluOpType

    with (
        nc.sbuf_tensor([P, BUFS * F], mybir.dt.float32) as x_sb,
        nc.sbuf_tensor([P, BUFS * F], mybir.dt.float32) as y_sb,
        nc.sbuf_tensor([P, RB * F], mybir.dt.float32) as r_sb,
        nc.sbuf_tensor([P, F], mybir.dt.float32) as dump_sb,
        nc.sbuf_tensor([P, 3 * ACC], mybir.dt.float32) as acc_sb,
        nc.sbuf_tensor([P, 1], mybir.dt.float32) as neg_sb,
        nc.semaphore() as rsem,
        nc.semaphore() as msem,
        nc.semaphore() as aasem,
        nc.semaphore() as dasem,
        nc.semaphore() as bsem,
        nc.semaphore() as osem,
    ):
        xsem_ctxs = [nc.semaphore(f"xsem_load{s}") for s in range(BUFS)]
        ysem_ctxs = [nc.semaphore(f"ysem_load{s}") for s in range(BUFS)]
        xsem = [c.__enter__() for c in xsem_ctxs]
        ysem = [c.__enter__() for c in ysem_ctxs]
        try:
            with nc.Block() as block:
                xs = lambda s, w: x_sb[:, s * F : s * F + w]
                ys = lambda s, w: y_sb[:, s * F : s * F + w]
                rs_ = lambda r_, w: r_sb[:, r_ * F : r_ * F + w]

                def dram(h, it):
                    i, o, w, _, _ = it
                    return h[i][:, o : o + w] if w != F else h[i]

                def wait_consumed(eng, j):
                    if is_dve(j):
                        eng.wait_ge(dasem, ndve(j))
                    else:
                        eng.wait_ge(aasem, nact[j + 1])

                @block.sync
                def _(sync):
                    for k in range(T):
                        it, s = items[k], k % BUFS
                        if k >= BUFS:
                            wait_consumed(sync, k - BUFS)
                        sync.dma_start(out=xs(s, it[2]), in_=dram(x_h, it)).then_inc(
                            xsem[s], 16
                        )
                        if k >= BUFS:
                            sync.wait_ge(rsem, k - BUFS + 1)
                        sync.dma_start(out=ys(s, it[2]), in_=dram(y_h, it)).then_inc(
                            ysem[s], 16
                        )
                    sync.wait_ge(aasem, nact[T])
                    if T - nact[T] > 0:
                        sync.wait_ge(dasem, T - nact[T])
                    sync.dma_start(out=out_h[:], in_=acc_sb[:]).then_inc(osem, 16)
                    sync.wait_ge(osem, 16)

                @block.vector
                def _(vector):
                    vector.memset(neg_sb[:], -1.0)
                    vector.memset(acc_sb[:], 0.0).then_inc(bsem, 1)
                    for k in range(T):
                        it, s, r_ = items[k], k % BUFS, k % RB
                        w, col = it[2], it[3]
                        vector.wait_ge(xsem[s], 16 * (k // BUFS + 1))
                        vector.wait_ge(rsem, k + 1)
                        nc.vector.tensor_mul(
                            xs(s, w), xs(s, w), rs_(r_, w)
                        ).then_inc(msem, 1)
                        if it[4]:
                            nc.vector.tensor_scalar(
                                dump_sb[:, :w], xs(s, w), 1.0, None,
                                AF.max, AF.add,
                                accum_out=acc_sb[:, ACC + col : ACC + col + 1],
                            )
                            nc.vector.tensor_scalar(
                                dump_sb[:, :w], xs(s, w), 1.0, None,
                                AF.min, AF.add,
                                accum_out=acc_sb[
                                    :, 2 * ACC + col : 2 * ACC + col + 1
                                ],
                            ).then_inc(dasem, 1)

                @block.scalar
                def _(scalar):
                    scalar.wait_ge(bsem, 1)

                    def do_abs(j):
                        it, sj = items[j], j % BUFS
                        w, col = it[2], it[3]
                        scalar.wait_ge(msem, j + 1)
                        nc.scalar.activation(
                            out=xs(sj, w),
                            in_=xs(sj, w),
                            func=AFT.Abs,
                            bias=neg_sb[:, 0:1],
                            scale=1.0,
                            accum_out=acc_sb[:, col : col + 1],
                        ).then_inc(aasem, 1)

                    for k in range(T):
                        it, s, r_ = items[k], k % BUFS, k % RB
                        scalar.wait_ge(ysem[s], 16 * (k // BUFS + 1))
                        if k >= RB:
                            scalar.wait_ge(msem, k - RB + 1)
                        _act_reciprocal(nc, rs_(r_, it[2]), ys(s, it[2])).then_inc(
                            rsem, 1
                        )
                        if k >= 1 and not is_dve(k - 1):
                            do_abs(k - 1)
                    if T > 0 and not is_dve(T - 1):
                        do_abs(T - 1)
        finally:
            for c in reversed(xsem_ctxs + ysem_ctxs):
                c.__exit__(None, None, None)
    return nc


# Selected variant for kernel() and the bench. "v6sp": x+y DMA issue on the
# SP ring, ACT recip (+13/16 of abs+accum), DVE mult (+3/16 of the abs work
# as max/min tensor_scalar pairs). See build_nc_v6; the R=1 NEFF that
# kernel() executes additionally splits the last tile (build_nc_v6ts) to
# shrink the single-shot drain.
VARIANT = "v6sp"


def build_kernel_nc(R=1):
    if VARIANT == "v8":
        return build_nc_v6(R=R, y_on_sp=True, z16=True, BUFS=11, RB=3)
    if VARIANT == "v6sp":
        if R == 1:
            # single-shot NEFF: split the last tile to shrink the drain
            return build_nc_v6ts(R=1, TS=4)
        return build_nc_v6(R=R, y_on_sp=True)
    if VARIANT == "v6":
        return build_nc_v6(R=R, y_on_sp=False)
    if VARIANT == "v4":
        return build_nc_v4(R=R)
    if VARIANT == "v4sp":
        return build_nc_v4(R=R, y_on_sp=True)
    return build_nc_dual(early_yload=True, tail_split=4, R=R)


def reduce_partials(partials):
    """Host-side reduction of one core's partials to a float64 sum."""
    p = partials.astype(np.float64)
    if VARIANT.startswith("v6") or VARIANT == "v8":
        NTC = p.shape[-1] // 3
        return p[..., :NTC].sum() + p[..., NTC : 2 * NTC].sum() - p[..., 2 * NTC :].sum()
    return p.sum()


def kernel(predictions, labels):
    global last_results
    preds = np.ascontiguousarray(np.asarray(predictions, dtype=np.float32)).reshape(
        N_CORES, NT, P, F
    )
    labs = np.ascontiguousarray(np.asarray(labels, dtype=np.float32)).reshape(
        N_CORES, NT, P, F
    )
    in_maps = [{"predictions": preds[c], "labels": labs[c]} for c in range(N_CORES)]
    nc = build_kernel_nc()
    last_results = run_bass_kernel_spmd(nc, in_maps, core_ids=list(range(N_CORES)))
    total = 0.0
    for r in last_results.results:
        total += reduce_partials(r["partials"])
    return np.float32(total / N_TOTAL * 100.0)

